# revision 67
# baseline (speedup 1.0000x reference)
"""Trainium2 Bass kernel for nn_CrossAttention (B=2, N=1024, L=4096, C=1024, H=16).

Sharding: head-parallel across 8 NeuronCores (2 heads per core).

Program A (per core, bf16 data / f32 PSUM accumulate):
  - q/k projections for the core's 2 heads (weights stationary, N=512
    streams); v is projected directly into natural [L, D] layout (the y block
    is the stationary operand, Wv the moving one), so no transposes are needed
  - full NxL attention for the 2 heads: scores (K=64, the two heads land in
    disjoint PE row-groups), exp via ACT, AV accumulation with an appended
    ones column producing the softmax denominator, reciprocal-broadcast
    normalize
  - output: ahat (the 128 local channels of softmax(qk)v, transposed layout)
    written shard-major for an all-to-all: a2a_out[j] = ahat[:, j*256:(j+1)*256]

Host: pure all-to-all shuffle of the 8 x 8 [128, 256] bf16 blocks (no math).

Program B (per core): full out-projection for a 256-row shard:
  out[r, :] = sum_ch A[r, ch] * Wp[:, ch]^T + bias, f32 output.

Both programs run their whole body inside a dynamic For_i(0, niter) loop with
niter read from a [1,1] int32 input, so a single NEFF execution can repeat the
kernel N times -- the only way to measure device time through the axon tunnel
(one bass_exec per program dispatch; ~70-100ms tunnel round trip per call).
Grading path passes niter=1.
"""

import functools

import numpy as np

B, N, L, C = 2, 1024, 4096, 1024
H, D = 16, 64
SCALE = D ** -0.5
NCORES = 8
LOCD = C // NCORES       # 128 local head-dims per core (2 heads x 64)
R = B * N                # 2048 query rows
RL = B * L               # 8192 key rows
ROWS_B = R // NCORES     # 256 output rows per core
KT = C // 128            # 8 contraction chunks
LT = RL // 128           # 64 key tiles of 128


def _split_excess_waits(nc, max_waits=1):
    """walrus in this container rejects >1 sync wait per instruction; hoist
    excess waits onto NoOps inserted before the offender on the same engine."""
    import concourse.mybir as mybir

    ctr = 0
    for fn in nc.m.functions:
        for blk in fn.blocks:
            insts = list(blk.instructions)
            new_insts = []
            changed = False
            for ins in insts:
                si = getattr(ins, "sync_info", None)
                if si is not None and si.on_wait and len(si.on_wait) > max_waits:
                    waits = list(si.on_wait)
                    excess, keep = waits[:-max_waits], waits[-max_waits:]
                    for i in range(0, len(excess), max_waits):
                        ctr += 1
                        nop = mybir.InstNoOp(
                            name=f"waitsplit_{ctr}",
                            engine=ins.engine,
                            sync_info=mybir.SyncInfo(
                                on_wait=excess[i : i + max_waits], on_update=[]
                            ),
                            text_hint="waitsplit",
                        )
                        new_insts.append(nop)
                        nc.register_instruction(nop, overwrite=True)
                    ins.sync_info = mybir.SyncInfo(
                        on_wait=keep, on_update=list(si.on_update)
                    )
                    changed = True
                new_insts.append(ins)
            if changed:
                blk.instructions = new_insts


@functools.cache
def _build_a(st_pair=False, hints=False, qfirst=True, bi=False, direct_mul=False,
             sim_once=False):
    import contextlib

    import concourse.bass as bass
    import concourse.mybir as mybir
    import concourse.tile as tile

    f32 = mybir.dt.float32
    f32r = mybir.dt.float32r
    bf16 = mybir.dt.bfloat16
    i32 = mybir.dt.int32

    nc = bass.Bass()

    niter = nc.declare_dram_parameter("niter", [1, 1], i32, isOutput=False)
    xT = nc.declare_dram_parameter("xT", [C, R], bf16, isOutput=False)
    yT = nc.declare_dram_parameter("yT", [C, RL], bf16, isOutput=False)
    wqT = nc.declare_dram_parameter("wqT", [C, LOCD], bf16, isOutput=False)
    wkT = nc.declare_dram_parameter("wkT", [C, LOCD], bf16, isOutput=False)
    wvT = nc.declare_dram_parameter("wvT", [C, LOCD], bf16, isOutput=False)
    onesf = nc.declare_dram_parameter("onesf", [1, 64], f32r, isOutput=False)
    onesh = nc.declare_dram_parameter("onesh", [128, 128], bf16, isOutput=False)
    identm = nc.declare_dram_parameter("identm", [128, 128], f32r, isOutput=False)
    a2a_out = nc.declare_dram_parameter(
        "a2a_out", [NCORES, 128, ROWS_B], bf16, isOutput=True
    )

    xTr = xT.rearrange("(kt p) c -> p kt c", p=128)
    yTr = yT.rearrange("(kt p) c -> p kt c", p=128)
    a2a_r = a2a_out.rearrange("j p c -> p j c")

    with tile.TileContext(nc) as tc:
        with (
            tc.tile_pool(name="boot", bufs=1) as boot,
            tc.tile_pool(name="const", bufs=1) as constp,
            tc.tile_pool(name="wts", bufs=1) as wpool,
            tc.tile_pool(name="stand", bufs=1) as stand,
            tc.tile_pool(name="src", bufs=4) as srcp,
            tc.tile_pool(name="vt", bufs=2) as vtp,
            tc.tile_pool(name="pt", bufs=4) as ptp,
            tc.tile_pool(name="small", bufs=2) as smallp,
            tc.tile_pool(name="psProj", bufs=2, space="PSUM") as psP,
            tc.tile_pool(name="psS", bufs=1 if st_pair else 2, space="PSUM") as psS,
            tc.tile_pool(name="psV", bufs=1, space="PSUM") as psV,
        ):
            if not sim_once:
                nit_sb = boot.tile([1, 1], i32)
                nc.sync.dma_start(nit_sb[:], niter[:])
                _, (nval,) = nc.values_load_multi_w_load_instructions(
                    nit_sb[0:1, 0:1], min_val=1, max_val=1_000_000,
                    skip_runtime_bounds_check=True,
                )

            hint = tuple(mybir.ALL_ENGINES) if hints else ()
            with (contextlib.nullcontext() if sim_once
                  else tc.For_i(0, nval, hint_engines=hint)):
                # ---- constants / weights ----
                ones1 = constp.tile([1, 64], f32r, tag="ones1")
                nc.sync.dma_start(ones1[:], onesf[:])
                ones_sb = constp.tile([128, 128], bf16, tag="onesh")
                nc.sync.dma_start(ones_sb[:], onesh[:])

                wq_s = wpool.tile([128, KT, LOCD], bf16, tag="wq")
                wk_s = wpool.tile([128, KT, LOCD], bf16, tag="wk")
                wv_s = wpool.tile([128, KT, LOCD], bf16, tag="wv")
                nc.sync.dma_start(wq_s[:], wqT.rearrange("(kt p) m -> p kt m", p=128))
                nc.sync.dma_start(wk_s[:], wkT.rearrange("(kt p) m -> p kt m", p=128))
                nc.sync.dma_start(wv_s[:], wvT.rearrange("(kt p) m -> p kt m", p=128))

                # ---- standing tensors ----
                qT_s = stand.tile([128, R], bf16, tag="qT")       # [locdim, (b,n)]
                kT_s = stand.tile([128, RL], bf16, tag="kT")      # [locdim, (b,l)]
                v_s = stand.tile([128, LT, 130], bf16, tag="v")   # [l%128, LT, 2x65]
                ahat_s = stand.tile([128, R], bf16, tag="ahat")   # [locdim, (b,n)]

                # ones columns of v_aug (col 64 of each head's 65-wide group)
                ones_cols = v_s[:, :, 0:130].rearrange(
                    "p t (a c) -> p t a c", a=2, c=65
                )[:, :, :, 64:65]
                nc.vector.tensor_copy(
                    out=ones_cols,
                    in_=ones_sb[:].rearrange(
                        "p (t a one) -> p t a one", t=64, a=2, one=1
                    ),
                )

                def emit_q(units):
                    for u in units:
                        src = srcp.tile([128, KT, 512], bf16, tag="src",
                                        name=f"xs{u}")
                        nc.sync.dma_start(
                            src[:], xTr[:, :, u * 512 : (u + 1) * 512]
                        )
                        acc = psP.tile([128, 512], f32, tag="proj",
                                       name=f"qacc{u}")
                        for kt in range(KT):
                            nc.tensor.matmul(
                                acc[:],
                                lhsT=(wq_s[:, kt, :]),
                                rhs=(src[:, kt, :]),
                                start=(kt == 0),
                                stop=(kt == KT - 1),
                            )
                        nc.vector.tensor_copy(
                            out=qT_s[:, u * 512 : (u + 1) * 512], in_=acc[:]
                        )

                def emit_kv(b):
                    for u in range(L // 512):
                        off = b * L + u * 512
                        src = srcp.tile(
                            [128, KT, 512], bf16, tag="src", name=f"ys{b}_{u}"
                        )
                        nc.sync.dma_start(src[:], yTr[:, :, off : off + 512])
                        kacc = psP.tile([128, 512], f32, tag="proj",
                                        name=f"kacc{b}_{u}")
                        for kt in range(KT):
                            nc.tensor.matmul(
                                kacc[:],
                                lhsT=(wk_s[:, kt, :]),
                                rhs=(src[:, kt, :]),
                                start=(kt == 0),
                                stop=(kt == KT - 1),
                            )
                        nc.vector.tensor_copy(
                            out=kT_s[:, off : off + 512], in_=kacc[:]
                        )
                        # v natural: y block stationary, Wv moving
                        vps = psP.tile(
                            [128, 4, 128], f32, tag="proj", name=f"vps{b}_{u}"
                        )
                        for j in range(4):
                            for kt in range(KT):
                                nc.tensor.matmul(
                                    vps[:, j, :],
                                    lhsT=(src[:, kt, j * 128 : (j + 1) * 128]),
                                    rhs=(wv_s[:, kt, :]),
                                    start=(kt == 0),
                                    stop=(kt == KT - 1),
                                )
                        for j in range(4):
                            t = off // 128 + j
                            nc.vector.tensor_copy(
                                out=v_s[:, t, 0:130].rearrange(
                                    "p (a c) -> p a c", a=2, c=65
                                )[:, :, 0:64],
                                in_=vps[:, j, :].rearrange(
                                    "p (a c) -> p a c", a=2, c=64
                                ),
                            )

                def emit_attn(b):
                    nlt = 2 if st_pair else 1
                    for u in range(2):
                        ncol = b * N + u * 512
                        av = psV.tile([128, 2, 512], f32, tag="av",
                                      name=f"av{b}_{u}")
                        for g in range(32 // nlt):
                            st = psS.tile(
                                [128, nlt, 2, 512], f32, tag="st",
                                name=f"st{b}_{u}_{g}"
                            )
                            for j in range(nlt):
                                koff = b * L + (g * nlt + j) * 128
                                for h in range(2):
                                    nc.tensor.matmul(
                                        st[:, j, h, :],
                                        lhsT=(
                                            kT_s[h * 64 : (h + 1) * 64,
                                                 koff : koff + 128]
                                        ),
                                        rhs=(
                                            qT_s[h * 64 : (h + 1) * 64,
                                                 ncol : ncol + 512]
                                        ),
                                        start=True,
                                        stop=True,
                                    )
                            pt = ptp.tile([128, nlt, 2, 512], bf16, tag="pt",
                                          name=f"pt{b}_{u}_{g}")
                            nc.scalar.activation(
                                pt[:], st[:], mybir.ActivationFunctionType.Exp,
                                scale=SCALE,
                            )
                            for j in range(nlt):
                                lt = g * nlt + j
                                for h in range(2):
                                    nc.tensor.matmul(
                                        av[0:65, h, :],
                                        lhsT=(
                                            v_s[:, b * 32 + lt,
                                                h * 65 : h * 65 + 65]
                                        ),
                                        rhs=(pt[:, j, h, :]),
                                        start=(lt == 0),
                                        stop=(lt == 31),
                                    )
                        for h in range(2):
                            recip = smallp.tile([1, 512], f32r, tag="recip",
                                                name=f"rc{b}_{u}_{h}")
                            with nc.allow_low_precision(
                                reason="f32r reciprocal feeds f32r broadcast"
                            ):
                                nc.vector.reciprocal(recip[:], av[64:65, h, :])
                            bc_ps = psP.tile([128, 512], f32, tag="proj",
                                             name=f"bp{b}_{u}_{h}")
                            nc.tensor.matmul(
                                bc_ps[0:64, :],
                                lhsT=ones1[:],
                                rhs=recip[:],
                                start=True,
                                stop=True,
                            )
                            if direct_mul:
                                nc.vector.tensor_mul(
                                    out=ahat_s[h * 64 : (h + 1) * 64,
                                               ncol : ncol + 512],
                                    in0=av[0:64, h, :],
                                    in1=bc_ps[0:64, :],
                                )
                            else:
                                bcst = smallp.tile([64, 512], f32r, tag="bcst",
                                                   name=f"bc{b}_{u}_{h}")
                                nc.vector.tensor_copy(out=bcst[:], in_=bc_ps[0:64, :])
                                nc.vector.tensor_mul(
                                    out=ahat_s[h * 64 : (h + 1) * 64,
                                               ncol : ncol + 512],
                                    in0=av[0:64, h, :],
                                    in1=bcst[:],
                                )
                        # drain this block's 2 output shards right away
                        j0 = (b * N + u * 512) // ROWS_B
                        nc.sync.dma_start(
                            a2a_r[:, j0 : j0 + 2, :],
                            ahat_s[:, ncol : ncol + 512].rearrange(
                                "p (j c) -> p j c", j=2
                            ),
                        )

                if bi:
                    # batch-interleaved: attention b0 outranks b1 projections
                    emit_q([0, 1])
                    emit_kv(0)
                    emit_attn(0)
                    emit_q([2, 3])
                    emit_kv(1)
                    emit_attn(1)
                elif qfirst:
                    emit_q([0, 1])
                    emit_kv(0)
                    emit_q([2, 3])
                    emit_kv(1)
                    emit_attn(0)
                    emit_attn(1)
                else:
                    emit_q([0, 1, 2, 3])
                    emit_kv(0)
                    emit_kv(1)
                    emit_attn(0)
                    emit_attn(1)

    _split_excess_waits(nc)
    return nc


@functools.cache
def _build_a_pipe(sim_once=False):
    """Software-pipelined program A.

    Four attention segments (b,uq), each 32 g-steps of [exp(g) | filler |
    st(g+1) | av(g-1)]: av lags one step so it never waits on exp; filler
    thunks (kv-projection blocks, q-projection blocks) are spread across
    segments to keep PE busy under ACT-bound stretches, with kv(0) streamed
    into segment 1 at a one-block lag and kv(1) split across segments 2-3.
    Each segment's normalize is deferred into the next segment.
    PSUM: st 2buf x 2 banks + av 2 banks + proj 2 banks = 8.

    v is stored per head as [l, 128]: 64 value dims + 64 ones columns, so
    the AV matmul replicates the softmax denominator across psum partitions
    64..127 (same streaming cost) and normalize is a 64-lane reciprocal +
    multiply on DVE -- no PE broadcast matmul, no 1-lane reciprocal.
    """
    import contextlib
    from collections import deque

    import concourse.bass as bass
    import concourse.mybir as mybir
    import concourse.tile as tile

    f32 = mybir.dt.float32
    f32r = mybir.dt.float32r
    bf16 = mybir.dt.bfloat16
    i32 = mybir.dt.int32

    nc = bass.Bass()

    niter = nc.declare_dram_parameter("niter", [1, 1], i32, isOutput=False)
    xT = nc.declare_dram_parameter("xT", [C, R], bf16, isOutput=False)
    yT = nc.declare_dram_parameter("yT", [C, RL], bf16, isOutput=False)
    wqT = nc.declare_dram_parameter("wqT", [C, LOCD], bf16, isOutput=False)
    wkT = nc.declare_dram_parameter("wkT", [C, LOCD], bf16, isOutput=False)
    wvT = nc.declare_dram_parameter("wvT", [C, LOCD], bf16, isOutput=False)
    a2a_out = nc.declare_dram_parameter(
        "a2a_out", [NCORES, 128, ROWS_B], bf16, isOutput=True
    )

    xTr = xT.rearrange("(kt p) c -> p kt c", p=128)
    yTr = yT.rearrange("(kt p) c -> p kt c", p=128)
    a2a_r = a2a_out.rearrange("j p c -> p j c")

    with tile.TileContext(nc) as tc:
        with (
            tc.tile_pool(name="boot", bufs=1) as boot,
            tc.tile_pool(name="const", bufs=1) as constp,
            tc.tile_pool(name="wts", bufs=1) as wpool,
            tc.tile_pool(name="stand", bufs=1) as stand,
            tc.tile_pool(name="src", bufs=6) as srcp,
            tc.tile_pool(name="pt", bufs=6) as ptp,
            tc.tile_pool(name="small", bufs=2) as smallp,
            tc.tile_pool(name="avcp", bufs=2) as avcpp,
            tc.tile_pool(name="psProj", bufs=2, space="PSUM") as psP,
            tc.tile_pool(name="psS", bufs=2, space="PSUM") as psS,
            tc.tile_pool(name="psV", bufs=1, space="PSUM") as psV,
        ):
            if not sim_once:
                nit_sb = boot.tile([1, 1], i32)
                nc.sync.dma_start(nit_sb[:], niter[:])
                _, (nval,) = nc.values_load_multi_w_load_instructions(
                    nit_sb[0:1, 0:1], min_val=1, max_val=1_000_000,
                    skip_runtime_bounds_check=True,
                )

            # standing v tile lives outside the loop so its ones columns
            # (softmax-denominator trick) are initialized exactly once
            v_s = stand.tile([128, LT, 256], bf16, tag="v")
            ones_cols = v_s[:, :, 0:256].rearrange(
                "p t (a c) -> p t a c", a=2, c=128
            )[:, :, :, 64:128]
            nc.gpsimd.memset(ones_cols, 1.0)

            # loop-invariant weights: load once, on the ACT hwdge queue so
            # the per-iteration x/y stream (SP queue) isn't delayed
            wq_s = wpool.tile([128, KT, LOCD], bf16, tag="wq")
            wk_s = wpool.tile([128, KT, LOCD], bf16, tag="wk")
            wv_s = wpool.tile([128, KT, LOCD], bf16, tag="wv")
            nc.scalar.dma_start(wq_s[:], wqT.rearrange("(kt p) m -> p kt m", p=128))
            nc.scalar.dma_start(wk_s[:], wkT.rearrange("(kt p) m -> p kt m", p=128))
            nc.scalar.dma_start(wv_s[:], wvT.rearrange("(kt p) m -> p kt m", p=128))

            with (contextlib.nullcontext() if sim_once else tc.For_i(0, nval, staggered_reset=True)):
                def load_x(u, nsplit=2, eng=None):
                    src = srcp.tile([128, KT, 512], bf16, tag="src",
                                    name=f"xs{u}")
                    # split so the first matmuls start sooner
                    step = KT // nsplit
                    for s in range(0, KT, step):
                        (eng or nc.sync).dma_start(
                            src[:, s : s + step, :],
                            xTr[:, s : s + step, u * 512 : (u + 1) * 512],
                        )
                    return src

                def load_y(b, u, nsplit=1, eng=None):
                    off = b * L + u * 512
                    src = srcp.tile(
                        [128, KT, 512], bf16, tag="src", name=f"ys{b}_{u}"
                    )
                    step = KT // nsplit
                    for s in range(0, KT, step):
                        (eng or nc.sync).dma_start(
                            src[:, s : s + step, :],
                            yTr[:, s : s + step, off : off + 512],
                        )
                    return src

                xsrc0 = load_x(0, nsplit=2)
                ysrc00 = load_y(0, 0, nsplit=2)
                xsrc1 = load_x(1)
                ysrc01 = load_y(0, 1)

                # ---- standing tensors ----
                qT_s = stand.tile([128, R], bf16, tag="qT")
                kT_s = stand.tile([128, RL], bf16, tag="kT")
                ahat_s = stand.tile([128, R], bf16, tag="ahat")

                def emit_q(u, src=None):
                    if src is None:
                        src = load_x(u)
                    acc = psP.tile([128, 512], f32, tag="proj",
                                   name=f"qacc{u}")
                    for kt in range(KT):
                        nc.tensor.matmul(
                            acc[:],
                            lhsT=(wq_s[:, kt, :]),
                            rhs=(src[:, kt, :]),
                            start=(kt == 0),
                            stop=(kt == KT - 1),
                        )
                    nc.vector.tensor_copy(
                        out=qT_s[:, u * 512 : (u + 1) * 512], in_=acc[:]
                    )

                def emit_kv_block(b, u, src=None):
                    off = b * L + u * 512
                    if src is None:
                        src = load_y(b, u)
                    kacc = psP.tile([128, 512], f32, tag="proj",
                                    name=f"kacc{b}_{u}")
                    for kt in range(KT):
                        nc.tensor.matmul(
                            kacc[:],
                            lhsT=(wk_s[:, kt, :]),
                            rhs=(src[:, kt, :]),
                            start=(kt == 0),
                            stop=(kt == KT - 1),
                        )
                    nc.vector.tensor_copy(
                        out=kT_s[:, off : off + 512], in_=kacc[:]
                    )
                    vps = psP.tile(
                        [128, 4, 128], f32, tag="proj", name=f"vps{b}_{u}"
                    )
                    for j in range(4):
                        for kt in range(KT):
                            nc.tensor.matmul(
                                vps[:, j, :],
                                lhsT=(src[:, kt, j * 128 : (j + 1) * 128]),
                                rhs=(wv_s[:, kt, :]),
                                start=(kt == 0),
                                stop=(kt == KT - 1),
                            )
                    for j in range(4):
                        t = off // 128 + j
                        nc.vector.tensor_copy(
                            out=v_s[:, t, 0:256].rearrange(
                                "p (a c) -> p a c", a=2, c=128
                            )[:, :, 0:64],
                            in_=vps[:, j, :].rearrange(
                                "p (a c) -> p a c", a=2, c=64
                            ),
                        )

                def attn_seg(b, uq, fillers, epi_prev, norm_prev):
                    """Pipelined attention for query block (b, uq).

                    fillers: deque of (g_due, thunk) PE-work emitted once
                    g reaches g_due (g_due=-1: before st(0)).
                    epi_prev: previous segment's av(31)+CAST, emitted after
                    this segment's st(0) so the PE issues new scores while
                    ACT drains the old segment's last exps.
                    norm_prev: deferred normalize of the previous segment,
                    emitted a few g-steps in so its PE/DVE ops hide under
                    this segment's stream.
                    """
                    ncol = b * N + uq * 512
                    av = psV.tile([128, 2, 512], f32, tag="av",
                                  name=f"av{b}_{uq}")
                    sts = {}
                    pts = {}

                    def pop_due(g):
                        while fillers and fillers[0][0] <= g:
                            fillers.popleft()[1]()

                    def emit_st(g):
                        st = psS.tile([128, 2, 512], f32, tag="st",
                                      name=f"st{b}_{uq}_{g}")
                        koff = b * L + g * 128
                        for h in range(2):
                            nc.tensor.matmul(
                                st[:, h, :],
                                lhsT=(kT_s[h * 64 : (h + 1) * 64,
                                           koff : koff + 128]),
                                rhs=(qT_s[h * 64 : (h + 1) * 64,
                                          ncol : ncol + 512]),
                                start=True,
                                stop=True,
                            )
                        sts[g] = st

                    def emit_av(g):
                        pt = pts.pop(g)
                        for h in range(2):
                            nc.tensor.matmul(
                                av[:, h, :],
                                lhsT=(v_s[:, b * 32 + g,
                                          h * 128 : h * 128 + 128]),
                                rhs=(pt[:, h, :]),
                                start=(g == 0),
                                stop=(g == 31),
                            )

                    pop_due(-1)
                    emit_st(0)
                    for g in range(32):
                        pt = ptp.tile([128, 2, 512], bf16, tag="pt",
                                      name=f"pt{b}_{uq}_{g}")
                        nc.scalar.activation(
                            pt[:], sts.pop(g)[:],
                            mybir.ActivationFunctionType.Exp, scale=SCALE,
                        )
                        pts[g] = pt
                        if g == 0 and epi_prev is not None:
                            epi_prev()
                        if g == 2 and norm_prev is not None:
                            norm_prev()
                        pop_due(g)
                        if g < 31:
                            emit_st(g + 1)
                        if g >= 1:
                            emit_av(g - 1)

                    # deferred epilogue: last av + one fast CAST that parks
                    # av in SBUF (bf16: 2x DVE rate and ahat is bf16 anyway)
                    # so the psum bank frees; the reciprocal runs later
                    avcp = avcpp.tile([128, 2, 512], bf16, tag="avcp",
                                      name=f"avcp{b}_{uq}")

                    def epi():
                        emit_av(31)
                        nc.vector.tensor_copy(out=avcp[:], in_=av[:])

                    def norm(act_recip=False):
                        for h in range(2):
                            rec = smallp.tile([64, 512], bf16, tag="rec",
                                              name=f"rc{b}_{uq}_{h}")
                            with nc.allow_low_precision(
                                reason="softmax denominators are O(1e3) sums"
                            ):
                                if act_recip:
                                    # 1/d = exp(-ln d) on ACT: frees the
                                    # tail from the 3.3us DVE reciprocal
                                    lg = smallp.tile(
                                        [64, 512], f32, tag="lg",
                                        name=f"lg{b}_{uq}_{h}")
                                    nc.scalar.activation(
                                        lg[:], avcp[64:128, h, :],
                                        mybir.ActivationFunctionType.Ln,
                                    )
                                    nc.scalar.activation(
                                        rec[:], lg[:],
                                        mybir.ActivationFunctionType.Exp,
                                        scale=-1.0,
                                    )
                                else:
                                    nc.vector.reciprocal(
                                        rec[:], avcp[64:128, h, :]
                                    )
                                nc.vector.tensor_mul(
                                    out=ahat_s[h * 64 : (h + 1) * 64,
                                               ncol : ncol + 512],
                                    in0=avcp[0:64, h, :],
                                    in1=rec[:],
                                )
                        j0 = ncol // ROWS_B
                        # last segment's output DMA rides the scalar queue so
                        # SP's staggered back-edge isn't held by the tail --
                        # next iteration's x/y prefetch starts ~30us earlier
                        oeng = nc.scalar if (b, uq) == (1, 1) else nc.sync
                        oeng.dma_start(
                            a2a_r[:, j0 : j0 + 2, :],
                            ahat_s[:, ncol : ncol + 512].rearrange(
                                "p (j c) -> p j c", j=2
                            ),
                        )

                    return epi, norm

                emit_q(0, xsrc0)

                def kv(b, u, src=None):
                    return functools.partial(emit_kv_block, b, u, src)

                # x blocks 2/3 are prefetched by DMA-only fillers mid-seg1
                # so segments 2/3 never stall on their transfer
                srcs = {}

                def prefetch_x(u):
                    def f():
                        srcs[u] = load_x(u)
                    return f

                def emit_q_pref(u):
                    def f():
                        emit_q(u, srcs.get(u))
                    return f

                # seg1 (b0,u0): kv(0) stream at one-block lag; q1 (needed by
                # seg2) emitted after kv block 0 so the PE isn't FIFO-blocked
                # on xs1 while ys00 is already resident
                f1 = deque(
                    [(-1, kv(0, 0, ysrc00)),
                     (0, functools.partial(emit_q, 1, xsrc1)),
                     (0, kv(0, 1, ysrc01)),
                     (4, kv(0, 2)),
                     (8, prefetch_x(2)),
                     (8, kv(0, 3)),
                     (12, kv(0, 4)),
                     (16, prefetch_x(3)),
                     (16, kv(0, 5)),
                     (20, kv(0, 6)),
                     (24, kv(0, 7))]
                )
                # seg2 (b0,u1): q2 (needed by seg3) + first half of kv(1)
                f2 = deque(
                    [(0, emit_q_pref(2))]
                    + [(4 + 5 * u, kv(1, u)) for u in range(4)]
                )
                # seg3 (b1,u0): q3 (needed by seg4) + rest of kv(1);
                # kv1 block u covers key chunks 4u..4u+3, st(g+1) at step g
                # needs block (g+1)//4 -> block u due by g = 4u-1.
                f3 = deque(
                    [(0, emit_q_pref(3))]
                    + [(4 * (u - 4), kv(1, u)) for u in range(4, 8)]
                )
                # ACT-engine reciprocal for norms that land in PE-bound
                # segments (ACT has slack there) and for the tail; n3 lands
                # in ACT-bound seg4, so it stays on DVE.
                e1, n1 = attn_seg(0, 0, f1, None, None)
                e2, n2 = attn_seg(0, 1, f2, e1,
                                  functools.partial(n1, act_recip=True))
                e3, n3 = attn_seg(1, 0, f3, e2,
                                  functools.partial(n2, act_recip=True))
                e4, n4 = attn_seg(1, 1, deque(), e3, n3)
                e4()
                n4(act_recip=True)

    _split_excess_waits(nc)
    return nc


@functools.cache
def _build_b(sim_once=False):
    import contextlib

    import concourse.bass as bass
    import concourse.mybir as mybir
    import concourse.tile as tile

    f32 = mybir.dt.float32
    bf16 = mybir.dt.bfloat16
    i32 = mybir.dt.int32

    nc = bass.Bass()
    niter = nc.declare_dram_parameter("niter", [1, 1], i32, isOutput=False)
    a2ab = nc.declare_dram_parameter("a2ab", [128, NCORES, ROWS_B], bf16, isOutput=False)
    wpT = nc.declare_dram_parameter("wpT", [C, C], bf16, isOutput=False)
    biasb = nc.declare_dram_parameter("biasb", [128, C], f32, isOutput=False)
    out_shard = nc.declare_dram_parameter("out_shard", [ROWS_B, C], f32, isOutput=True)

    wpTr = wpT.rearrange("(kt p) c -> p kt c", p=128)

    with tile.TileContext(nc) as tc:
        with (
            tc.tile_pool(name="boot", bufs=1) as boot,
            tc.tile_pool(name="cn", bufs=1) as constp,
            tc.tile_pool(name="wp", bufs=8) as wpp,
            tc.tile_pool(name="sb", bufs=2) as pool,
            tc.tile_pool(name="ps", bufs=2, space="PSUM") as ps,
        ):
            if not sim_once:
                nit_sb = boot.tile([1, 1], i32)
                nc.sync.dma_start(nit_sb[:], niter[:])
                _, (nval,) = nc.values_load_multi_w_load_instructions(
                    nit_sb[0:1, 0:1], min_val=1, max_val=1_000_000,
                    skip_runtime_bounds_check=True,
                )
            # loop-invariant: out-projection weights + bias stay resident
            wp_t = []
            for i in range(KT):
                w = wpp.tile([128, C], bf16, tag="wp", name=f"wp{i}")
                nc.scalar.dma_start(w[:], wpTr[:, i, :])
                wp_t.append(w)
            bias_s = constp.tile([128, C], f32, tag="bias")
            nc.scalar.dma_start(bias_s[:], biasb[:])

            with (contextlib.nullcontext() if sim_once else tc.For_i(0, nval, staggered_reset=True)):
                # per-source-core chunks so matmuls chase the DMA stream
                a_s = constp.tile([128, NCORES, ROWS_B], bf16, tag="a2a")
                for i in range(NCORES):
                    nc.sync.dma_start(a_s[:, i, :], a2ab[:, i, :])
                # t-major: t=0 finishes early so its bias-add + store run
                # under t=1's matmuls
                for t in range(ROWS_B // 128):
                    acc = ps.tile([128, 2, 512], f32, tag="acc",
                                  name=f"acc{t}")
                    for i in range(KT):
                        for cb in range(2):
                            nc.tensor.matmul(
                                acc[:, cb, :],
                                lhsT=(a_s[:, i, t * 128 : (t + 1) * 128]),
                                rhs=(wp_t[i][:, cb * 512 : (cb + 1) * 512]),
                                start=(i == 0),
                                stop=(i == KT - 1),
                            )
                    osb = pool.tile([128, C], f32, tag="osb", name=f"osb{t}")
                    nc.vector.tensor_add(
                        out=osb[:].rearrange("p (a c) -> p a c", a=2, c=512),
                        in0=acc[:],
                        in1=bias_s[:].rearrange("p (a c) -> p a c", a=2, c=512),
                    )
                    nc.scalar.dma_start(
                        out_shard[t * 128 : (t + 1) * 128, :], osb[:]
                    )

    _split_excess_waits(nc)
    return nc


def _bf16(a):
    import ml_dtypes
    return np.ascontiguousarray(np.asarray(a, np.float32)).astype(ml_dtypes.bfloat16)


def _prep_inputs_a(x, y, Wq, Wk, Wv, niter=1):
    x = np.asarray(x, np.float32)
    y = np.asarray(y, np.float32)
    xT = _bf16(x.reshape(R, C).T)
    yT = _bf16(y.reshape(RL, C).T)
    onesf = np.ones((1, 64), np.float32)
    onesh = np.ones((128, 128), np.float32)
    ident = np.eye(128, dtype=np.float32)
    nit = np.array([[niter]], np.int32)
    in_maps = []
    for i in range(NCORES):
        sl = slice(i * LOCD, (i + 1) * LOCD)
        in_maps.append(
            {
                "niter": nit,
                "xT": xT,
                "yT": yT,
                "wqT": _bf16(np.asarray(Wq, np.float32)[sl, :].T),
                "wkT": _bf16(np.asarray(Wk, np.float32)[sl, :].T),
                "wvT": _bf16(np.asarray(Wv, np.float32)[sl, :].T),
                "onesf": onesf,
                "onesh": _bf16(onesh),
                "identm": ident,
            }
        )
    return in_maps


def _prep_inputs_b(a2a_blocks, Wp, bp, niter=1):
    """a2a_blocks[i] = core i's program-A output, [NCORES, 128, ROWS_B] bf16."""
    wpT = _bf16(np.asarray(Wp, np.float32).T)
    bias = np.ascontiguousarray(
        np.broadcast_to(np.asarray(bp, np.float32), (128, C))
    )
    nit = np.array([[niter]], np.int32)
    in_maps = []
    for j in range(NCORES):
        a2ab = np.ascontiguousarray(
            np.stack([a2a_blocks[i][j] for i in range(NCORES)]).transpose(1, 0, 2)
        )
        in_maps.append({"niter": nit, "a2ab": a2ab, "wpT": wpT, "biasb": bias})
    return in_maps


def build_a_current():
    """The program-A builder in use (kernel() and test.py share it)."""
    return _build_a_pipe()


def kernel(x, y, Wq, Wk, Wv, Wp, bp):
    from concourse.bass_utils import run_bass_kernel_spmd

    nca = build_a_current()
    in_maps = _prep_inputs_a(x, y, Wq, Wk, Wv)
    res = run_bass_kernel_spmd(nca, in_maps, list(range(NCORES)))
    a2a = [res.results[i]["a2a_out"] for i in range(NCORES)]

    ncb = _build_b()
    in_maps2 = _prep_inputs_b(a2a, Wp, bp)
    res2 = run_bass_kernel_spmd(ncb, in_maps2, list(range(NCORES)))
    out = np.concatenate(
        [res2.results[j]["out_shard"] for j in range(NCORES)], axis=0
    )
    return out.reshape(B, N, C).astype(np.float32)

